# revision 5
# baseline (speedup 1.0000x reference)
"""Multi-head self-attention (B=4, S=2048, D=1024, H=16) on 8 trn2 NeuronCores.

Sharding: core c -> batch b = c//2, head-group g = c%2 (8 heads = 512 of the
1024 QKV/output columns). Host does layout prep (x transpose, W column slices)
and the final gather/transpose - no collectives.

Per-core pipeline:
  phase 1 (f32r matmuls, fp32 psum): QT/KT [512,2048] and V, rounded to fp16
    on the psum->sbuf copy. V packed as Vx[128,16,8,65] with a ones column per
    head (PV row 64 = softmax denominator).
  phase 2 (fp16 matmuls): per head-pair, q-chunk(512), k-block(16):
    scoresT[k,q] psum fp32 <- KT-tile.T @ QT-chunk, two heads row-packed into
    one [128,1024] psum tile via tile_position (0,0)/(64,0).
    exp(s/16 - 4): even k-blocks directly on ACT (psum->sbuf, per-bank-split
    into 512-elem instructions); odd k-blocks DVE-copied to SBUF fp16 and
    exp'd in two big [128,4096] ACT ops (amortizes ACT per-op overhead;
    balances ACT vs DVE).  PV pv[65,512] psum += Vx.T @ expt accumulates
    out-of-order over k-blocks (safe: same-tile WAW keeps PE order; start on
    the first, stop on the last emitted).
  normalize: den -> sbuf copy -> reciprocal -> gpsimd partition_broadcast ->
    DVE multiply into outT[512,2048] fp32; DMA out per head-pair.
"""
import numpy as np

import concourse.bacc as bacc
import concourse.mybir as mybir
import concourse.tile as tile
from concourse.bass_utils import run_bass_kernel_spmd

B, S, D, H = 4, 2048, 1024, 16
DH = D // H            # 64
NCORES = 8
HLOC = H // 2          # 8 heads per core
DLOC = HLOC * DH       # 512 output cols per core
F32 = mybir.dt.float32
F32R = mybir.dt.float32r
F16 = mybir.dt.float16
EXPF = mybir.ActivationFunctionType.Exp
SHIFT = 4.0            # exp(s/H - SHIFT): keeps exp in fp16 range (smax ~11.3)

SC = 512               # s-chunk in phase 1
QC = 512               # q-chunk in phase 2
NKB = S // 128         # 16 k-blocks
NDT = D // 128         # 8 contraction tiles for QKV
DVE_KBS = tuple(range(1, NKB, 2))   # k-blocks exp'd via DVE-copy + batched ACT
BATCH = 2              # odd k-blocks per batched exp op


def _build():
    nc = bacc.Bacc("TRN2", target_bir_lowering=False, debug=False, num_devices=NCORES)
    xT = nc.dram_tensor("xT", [D, S], F32R, kind="ExternalInput").ap()
    Wq = nc.dram_tensor("Wq", [D, DLOC], F32R, kind="ExternalInput").ap()
    Wk = nc.dram_tensor("Wk", [D, DLOC], F32R, kind="ExternalInput").ap()
    Wv = nc.dram_tensor("Wv", [D, DLOC], F32R, kind="ExternalInput").ap()
    out = nc.dram_tensor("outT", [DLOC, S], F32, kind="ExternalOutput").ap()

    xT_t = xT.rearrange("(o p) s -> p o s", p=128)        # [128, 8, 2048]
    out_t = out.rearrange("(o p) s -> p o s", p=128)      # [128, 4, 2048]

    with tile.TileContext(nc) as tc:
        with tc.tile_pool(name="persist", bufs=1) as keep:
            qt = keep.tile([128, DLOC // 128, S], F32R)   # [p, 4, 2048]
            kt = keep.tile([128, DLOC // 128, S], F32R)
            vx = keep.tile([128, NKB, HLOC, DH + 1], F32R)
            bias_t = keep.tile([128, 1], F32)
            nc.vector.memset(bias_t[:], -SHIFT)

            # ---------------- phase 1: QKV projections (f32r) ------------
            with nc.named_scope("qkv"), \
                 tc.tile_pool(name="p1w", bufs=1) as p1w, \
                 tc.tile_pool(name="p1x", bufs=2) as p1x, \
                 tc.tile_pool(name="p1ps", bufs=3, space="PSUM") as p1ps:
                wq_sb = p1w.tile([128, NDT, DLOC], F32R)
                wk_sb = p1w.tile([128, NDT, DLOC], F32R)
                wv_sb = p1w.tile([128, NDT, DLOC], F32R)
                nc.gpsimd.dma_start(wq_sb[:], Wq.rearrange("(o p) m -> p o m", p=128))
                nc.gpsimd.dma_start(wk_sb[:], Wk.rearrange("(o p) m -> p o m", p=128))
                nc.gpsimd.dma_start(wv_sb[:], Wv.rearrange("(o p) m -> p o m", p=128))
                ones_t = p1w.tile([128, NKB, HLOC], F32)
                nc.vector.memset(ones_t[:], 1.0)
                nc.vector.tensor_copy(vx[:, :, :, DH], ones_t[:])

                xcs = []
                for sc in range(S // SC):
                    xc = p1x.tile([128, NDT, SC], F32R, tag="xc", name=f"xc{sc}")
                    nc.gpsimd.dma_start(xc[:], xT_t[:, :, sc * SC:(sc + 1) * SC])
                    xcs.append(xc)
                    # K first, then Q: attention can start before V finishes
                    for w_sb, dst in ((wk_sb, kt), (wq_sb, qt)):
                        for m in range(DLOC // 128):
                            ps = p1ps.tile([128, SC], F32, tag="qk", name="psqk")
                            for dt_i in range(NDT):
                                nc.tensor.matmul(
                                    ps[:],
                                    w_sb[:, dt_i, m * 128:(m + 1) * 128],
                                    xc[:, dt_i, :],
                                    start=(dt_i == 0), stop=(dt_i == NDT - 1),
                                )
                            nc.vector.tensor_copy(
                                dst[:, m, sc * SC:(sc + 1) * SC], ps[:])
                for sc in range(S // SC):
                    xc = xcs[sc]
                    for sb in range(SC // 128):
                        ps = p1ps.tile([128, DLOC], F32, tag="v", name="psv")
                        for dt_i in range(NDT):
                            nc.tensor.matmul(
                                ps[:],
                                xc[:, dt_i, sb * 128:(sb + 1) * 128],
                                wv_sb[:, dt_i, :],
                                start=(dt_i == 0), stop=(dt_i == NDT - 1),
                            )
                        s_idx = sc * (SC // 128) + sb
                        nc.vector.tensor_copy(
                            vx[:, s_idx, :, 0:DH],
                            ps[:].rearrange("p (h d) -> p h d", h=HLOC))

            # ---------------- phase 2: attention (fp16) ------------------
            with nc.named_scope("attn"), \
                 tc.tile_pool(name="p2o", bufs=1) as p2o, \
                 tc.tile_pool(name="p2e", bufs=2) as p2e, \
                 tc.tile_pool(name="p2b", bufs=2) as p2b, \
                 tc.tile_pool(name="p2n", bufs=2) as p2n, \
                 tc.tile_pool(name="ps_s", bufs=2, space="PSUM") as ps_s, \
                 tc.tile_pool(name="ps_pv", bufs=2, space="PSUM") as ps_pv:
                ot = p2o.tile([128, DLOC // 128, S], F32)
                for hp in range(HLOC // 2):
                    for qc in range(S // QC):
                        qs = slice(qc * QC, (qc + 1) * QC)
                        pv = [ps_pv.tile([DH + 1, QC], F32, tag=f"pv{h}",
                                         name=f"pv{h}") for h in range(2)]
                        scb = [p2b.tile([128, BATCH, 2 * QC], F32R, tag="scb",
                                        name=f"scb{i}")
                               for i in range(len(DVE_KBS) // BATCH)]
                        exb = [p2b.tile([128, BATCH, 2 * QC], F32R, tag="exb",
                                        name=f"exb{i}")
                               for i in range(len(DVE_KBS) // BATCH)]

                        def pv_mm(h, kb, rhs, start, stop):
                            nc.tensor.matmul(
                                pv[h][:], vx[:, kb, 2 * hp + h, :], rhs,
                                start=start, stop=stop, skip_group_check=True)

                        for kb in range(NKB):
                            ks = slice(kb * 128, (kb + 1) * 128)
                            sp = ps_s.tile([128, 2 * QC], F32, tag="sc", name="sp")
                            for h in range(2):
                                nc.tensor.matmul(
                                    sp[:, h * QC:(h + 1) * QC],
                                    kt[64 * h:64 * h + 64, hp, ks],
                                    qt[64 * h:64 * h + 64, hp, qs],
                                    start=True, stop=True,
                                    tile_position=(64 * h, 0))
                            if kb not in DVE_KBS:
                                ex = p2e.tile([128, 2 * QC], F32R, tag="ex",
                                              name="ex")
                                nc.scalar.activation(ex[:], sp[:], EXPF,
                                                     bias=bias_t[:],
                                                     scale=1.0 / H)
                                for h in range(2):
                                    pv_mm(h, kb, ex[:, h * QC:(h + 1) * QC],
                                          kb == 0, False)
                            else:
                                i = DVE_KBS.index(kb)
                                nc.vector.tensor_copy(
                                    scb[i // BATCH][:, i % BATCH, :], sp[:])
                                if i % BATCH == BATCH - 1:
                                    g = i // BATCH
                                    nc.scalar.activation(
                                        exb[g][:], scb[g][:], EXPF,
                                        bias=bias_t[:], scale=1.0 / H)
                                    for j in range(BATCH):
                                        okb = DVE_KBS[g * BATCH + j]
                                        for h in range(2):
                                            pv_mm(h, okb,
                                                  exb[g][:, j,
                                                         h * QC:(h + 1) * QC],
                                                  False, okb == DVE_KBS[-1])
                        for h in range(2):
                            dr = p2n.tile([1, QC], F32, tag=f"dr{h}",
                                          name=f"dr{h}")
                            nc.vector.tensor_copy(dr[:], pv[h][DH:DH + 1, :])
                            den = p2n.tile([1, QC], F32, tag=f"dn{h}",
                                           name=f"dn{h}")
                            nc.vector.reciprocal_approx_fast(den[:], dr[:])
                            bc = p2n.tile([DH, QC], F32, tag=f"bc{h}",
                                          name=f"bc{h}")
                            nc.gpsimd.partition_broadcast(bc[:], den[:])
                            nc.vector.tensor_mul(
                                ot[64 * h:64 * h + 64, hp, qs],
                                pv[h][0:DH, :], bc[:])
                    nc.gpsimd.dma_start(out_t[:, hp, :], ot[:, hp, :])
    nc.compile()
    return nc


def run(inputs, trace=False):
    x = np.asarray(inputs["encoder_input"], dtype=np.float32)
    Wq = np.asarray(inputs["Wq"], dtype=np.float32)
    Wk = np.asarray(inputs["Wk"], dtype=np.float32)
    Wv = np.asarray(inputs["Wv"], dtype=np.float32)

    nc = _build()
    in_maps = []
    for c in range(NCORES):
        b, g = c // 2, c % 2
        cols = slice(g * DLOC, (g + 1) * DLOC)
        in_maps.append({
            "xT": np.ascontiguousarray(x[b].T),
            "Wq": np.ascontiguousarray(Wq[:, cols]),
            "Wk": np.ascontiguousarray(Wk[:, cols]),
            "Wv": np.ascontiguousarray(Wv[:, cols]),
        })
    res = run_bass_kernel_spmd(nc, in_maps, core_ids=list(range(NCORES)),
                               trace=trace)
    out = np.empty((B, S, D), dtype=np.float32)
    for c in range(NCORES):
        b, g = c // 2, c % 2
        out[b, :, g * DLOC:(g + 1) * DLOC] = res.results[c]["outT"].T
    return out, res


def kernel(**inputs):
    out, _ = run(inputs, trace=False)
    return out


# revision 7
# speedup vs baseline: 1.2715x; 1.2715x over previous
"""Multi-head self-attention (B=4, S=2048, D=1024, H=16) on 8 trn2 NeuronCores.

Sharding: core c -> batch b = c//2, head-group g = c%2 (8 heads, 512 of the
1024 output/QKV columns). Each core computes Q/K/V projections for its slice
and full attention for its 8 heads. Host does layout prep (x transpose, W
column slices) and the final gather/transpose - no collectives needed.

Per-core device pipeline (all matmuls in float32r: full PE rate at N=512,
~1.6e-4 relative error):
  phase 1: QT[512,2048], KT[512,2048] = (W.T @ xT-chunks); V[2048,512] packed
           into Vx[128,16,8,65] with a ones column per head (PV denominator).
  phase 2: per head-pair, per q-chunk(512), per k-block(16):
           scoresT[k,q] psum <- KT-tile.T @ QT-chunk (2 heads -> [128,1024]);
           expT = ACT Exp(scores * 1/16) -> f32r sbuf;
           pv[65,512] psum += Vx-tile.T @ expT  (row 64 = softmax denominator)
           then normalize: out = pv[0:64] * partition_broadcast(1/pv[64]).
  output: outT[512,2048] per core; host writes out[b,:,cols] = outT.T.
"""
import numpy as np

import concourse.bacc as bacc
import concourse.mybir as mybir
import concourse.tile as tile
from concourse.bass_utils import run_bass_kernel_spmd

B, S, D, H = 4, 2048, 1024, 16
DH = D // H            # 64
NCORES = 8
HLOC = H // 2          # 8 heads per core
DLOC = HLOC * DH       # 512 output cols per core
F32 = mybir.dt.float32
F32R = mybir.dt.float32r
EXPF = mybir.ActivationFunctionType.Exp

SC = 512               # s-chunk in phase 1
QC = 512               # q-chunk in phase 2
NKB = S // 128         # 16 k-blocks
NDT = D // 128         # 8 contraction tiles for QKV


def _build():
    nc = bacc.Bacc("TRN2", target_bir_lowering=False, debug=False, num_devices=NCORES)
    xT = nc.dram_tensor("xT", [D, S], F32R, kind="ExternalInput").ap()
    Wq = nc.dram_tensor("Wq", [D, DLOC], F32R, kind="ExternalInput").ap()
    Wk = nc.dram_tensor("Wk", [D, DLOC], F32R, kind="ExternalInput").ap()
    Wv = nc.dram_tensor("Wv", [D, DLOC], F32R, kind="ExternalInput").ap()
    out = nc.dram_tensor("outT", [DLOC, S], F32, kind="ExternalOutput").ap()

    xT_t = xT.rearrange("(o p) s -> p o s", p=128)        # [128, 8, 2048]
    out_t = out.rearrange("(o p) s -> p o s", p=128)      # [128, 4, 2048]

    with tile.TileContext(nc) as tc:
        with tc.tile_pool(name="persist", bufs=1) as keep:
            qt = keep.tile([128, DLOC // 128, S], F32R)   # QT  [p, 4, 2048]
            kt = keep.tile([128, DLOC // 128, S], F32R)   # KT  [p, 4, 2048]
            vx = keep.tile([128, NKB, HLOC, DH + 1], F32R)

            # ---------------- phase 1: QKV projections -------------------
            with nc.named_scope("qkv"), \
                 tc.tile_pool(name="p1w", bufs=1) as p1w, \
                 tc.tile_pool(name="p1x", bufs=2) as p1x, \
                 tc.tile_pool(name="p1ps", bufs=3, space="PSUM") as p1ps:
                wq_sb = p1w.tile([128, NDT, DLOC], F32R)
                wk_sb = p1w.tile([128, NDT, DLOC], F32R)
                wv_sb = p1w.tile([128, NDT, DLOC], F32R)
                nc.gpsimd.dma_start(wq_sb[:], Wq.rearrange("(o p) m -> p o m", p=128))
                nc.gpsimd.dma_start(wk_sb[:], Wk.rearrange("(o p) m -> p o m", p=128))
                nc.gpsimd.dma_start(wv_sb[:], Wv.rearrange("(o p) m -> p o m", p=128))
                ones_t = p1w.tile([128, NKB, HLOC], F32)
                nc.vector.memset(ones_t[:], 1.0)
                nc.vector.tensor_copy(vx[:, :, :, DH], ones_t[:])

                xcs = []
                for sc in range(S // SC):
                    xc = p1x.tile([128, NDT, SC], F32R, tag="xc", name=f"xc{sc}")
                    nc.gpsimd.dma_start(xc[:], xT_t[:, :, sc * SC:(sc + 1) * SC])
                    xcs.append(xc)
                    for w_sb, dst in ((wk_sb, kt), (wq_sb, qt)):
                        for m in range(DLOC // 128):
                            ps = p1ps.tile([128, SC], F32, tag="qk")
                            for dt_i in range(NDT):
                                nc.tensor.matmul(
                                    ps[:],
                                    w_sb[:, dt_i, m * 128:(m + 1) * 128],
                                    xc[:, dt_i, :],
                                    start=(dt_i == 0), stop=(dt_i == NDT - 1),
                                )
                            nc.vector.tensor_copy(
                                dst[:, m, sc * SC:(sc + 1) * SC], ps[:])
                for sc in range(S // SC):
                    xc = p1x.tile([128, NDT, SC], F32R, tag="xc", name=f"xcv{sc}")
                    nc.gpsimd.dma_start(xc[:], xT_t[:, :, sc * SC:(sc + 1) * SC])
                    for sb in range(SC // 128):
                        ps = p1ps.tile([128, DLOC], F32, tag="v")
                        for dt_i in range(NDT):
                            nc.tensor.matmul(
                                ps[:],
                                xc[:, dt_i, sb * 128:(sb + 1) * 128],
                                wv_sb[:, dt_i, :],
                                start=(dt_i == 0), stop=(dt_i == NDT - 1),
                            )
                        s_idx = sc * (SC // 128) + sb
                        nc.vector.tensor_copy(
                            vx[:, s_idx, :, 0:DH],
                            ps[:].rearrange("p (h d) -> p h d", h=HLOC))

            # ---------------- phase 2: attention -------------------------
            with nc.named_scope("attn"), \
                 tc.tile_pool(name="p2o", bufs=1) as p2o, \
                 tc.tile_pool(name="p2e", bufs=3) as p2e, \
                 tc.tile_pool(name="p2n", bufs=2) as p2n, \
                 tc.tile_pool(name="ps_s", bufs=2, space="PSUM") as ps_s, \
                 tc.tile_pool(name="ps_pv", bufs=2, space="PSUM") as ps_pv:
                ot = p2o.tile([128, DLOC // 128, S], F32)
                for hp in range(HLOC // 2):
                    for qc in range(S // QC):
                        qs = slice(qc * QC, (qc + 1) * QC)
                        pv0 = ps_pv.tile([DH + 1, QC], F32, tag="pv0")
                        pv1 = ps_pv.tile([DH + 1, QC], F32, tag="pv1")
                        for kb in range(NKB):
                            ks = slice(kb * 128, (kb + 1) * 128)
                            sps = ps_s.tile([128, 2 * QC], F32, tag="sc")
                            nc.tensor.matmul(
                                sps[:, 0:QC],
                                kt[0:64, hp, ks], qt[0:64, hp, qs],
                                start=True, stop=True)
                            nc.tensor.matmul(
                                sps[:, QC:2 * QC],
                                kt[64:128, hp, ks], qt[64:128, hp, qs],
                                start=True, stop=True)
                            ex = p2e.tile([128, 2 * QC], F32R, tag="ex")
                            nc.scalar.activation(ex[:], sps[:], EXPF, scale=1.0 / H)
                            nc.tensor.matmul(
                                pv0[:], vx[:, kb, 2 * hp, :], ex[:, 0:QC],
                                start=(kb == 0), stop=(kb == NKB - 1),
                                skip_group_check=True)
                            nc.tensor.matmul(
                                pv1[:], vx[:, kb, 2 * hp + 1, :], ex[:, QC:2 * QC],
                                start=(kb == 0), stop=(kb == NKB - 1),
                                skip_group_check=True)
                        for half, pv in ((0, pv0), (1, pv1)):
                            dr = p2n.tile([1, QC], F32, tag="dr", name="dr")
                            nc.vector.tensor_copy(dr[:], pv[DH:DH + 1, :])
                            den = p2n.tile([1, QC], F32, tag="den", name="den")
                            nc.vector.reciprocal_approx_fast(den[:], dr[:])
                            bc = p2n.tile([DH, QC], F32, tag="bc", name="bc")
                            nc.gpsimd.partition_broadcast(bc[:], den[:])
                            nc.vector.tensor_mul(
                                ot[64 * half:64 * half + 64, hp, qs],
                                pv[0:DH, :], bc[:])
                    nc.gpsimd.dma_start(out_t[:, hp, :], ot[:, hp, :])

    nc.compile()
    return nc


def run(inputs, trace=False):
    x = np.asarray(inputs["encoder_input"], dtype=np.float32)
    Wq = np.asarray(inputs["Wq"], dtype=np.float32)
    Wk = np.asarray(inputs["Wk"], dtype=np.float32)
    Wv = np.asarray(inputs["Wv"], dtype=np.float32)

    nc = _build()
    in_maps = []
    for c in range(NCORES):
        b, g = c // 2, c % 2
        cols = slice(g * DLOC, (g + 1) * DLOC)
        in_maps.append({
            "xT": np.ascontiguousarray(x[b].T),
            "Wq": np.ascontiguousarray(Wq[:, cols]),
            "Wk": np.ascontiguousarray(Wk[:, cols]),
            "Wv": np.ascontiguousarray(Wv[:, cols]),
        })
    res = run_bass_kernel_spmd(nc, in_maps, core_ids=list(range(NCORES)),
                               trace=trace)
    out = np.empty((B, S, D), dtype=np.float32)
    for c in range(NCORES):
        b, g = c // 2, c % 2
        out[b, :, g * DLOC:(g + 1) * DLOC] = res.results[c]["outT"].T
    return out, res


def kernel(**inputs):
    out, _ = run(inputs, trace=False)
    return out
